# revision 1
# baseline (speedup 1.0000x reference)
"""DLRM on 8 Trainium2 NeuronCores (Bass/Tile), batch-split data-parallel.

Strategy: each core handles 1024 of the 8192 samples end-to-end; the 26
embedding tables are replicated to every core's HBM, so no collectives are
needed.  Per core: bottom MLP (bf16 matmuls, f32 PSUM) -> indirect-DMA
gather of embedding rows with inline f32->bf16 cast -> sum-pooling as PE
matmuls against a 0/1 pooling matrix (gives feature-major pooled vectors
directly) -> per-sample 27x27 gram matmuls -> DVE extraction into
32-aligned pair-chunk tiles -> top MLP with a host-transformed first-layer
weight (symmetric-half trick replaces the upper-triangle gather) ->
sigmoid -> [1024] f32.
"""
import sys
import numpy as np

if '/opt/trn_rl_repo' not in sys.path:
    sys.path.insert(0, '/opt/trn_rl_repo')

import ml_dtypes
import concourse.bass as bass
import concourse.mybir as mybir
import concourse.tile as tile
from concourse import bacc

F32 = mybir.dt.float32
BF16 = mybir.dt.bfloat16
I32 = mybir.dt.int32
AF = mybir.ActivationFunctionType

NT = 26          # tables
NF = 27          # features entering interaction
D = 128
P = 10           # pooling factor
VOCAB = 100000
N_TILES = 8      # 128-bag tiles per core
N_CORES = 8
TOPK0 = 1152     # padded z' rows (128 x + 7*128 pair chunks)
NCHUNK = 7


def build_nc(vocab: int = VOCAB, n_tiles: int = N_TILES, rounds: int = 1):
    BC = n_tiles * 128
    nc = bacc.Bacc(None, target_bir_lowering=False)

    emb = nc.dram_tensor("emb", [NT * vocab, D], F32, kind="ExternalInput")
    offs_d = nc.dram_tensor("offs", [128, n_tiles * NT * P], I32, kind="ExternalInput")
    xT_d = nc.dram_tensor("xT", [13, BC], BF16, kind="ExternalInput")
    pmat_d = nc.dram_tensor("pmat", [128, P, 128], BF16, kind="ExternalInput")
    wb0_d = nc.dram_tensor("wb0", [13, 512], BF16, kind="ExternalInput")
    wb1_d = nc.dram_tensor("wb1", [128, 4, 256], BF16, kind="ExternalInput")
    wb2_d = nc.dram_tensor("wb2", [128, 2, 128], BF16, kind="ExternalInput")
    tw0_d = nc.dram_tensor("tw0", [128, 9, 1024], BF16, kind="ExternalInput")
    tw1_d = nc.dram_tensor("tw1", [128, 8, 1024], BF16, kind="ExternalInput")
    tw2_d = nc.dram_tensor("tw2", [128, 8, 512], BF16, kind="ExternalInput")
    tw3_d = nc.dram_tensor("tw3", [128, 4, 256], BF16, kind="ExternalInput")
    tw4_d = nc.dram_tensor("tw4", [128, 2, 1], BF16, kind="ExternalInput")
    bias_d = nc.dram_tensor("bias", [128, 30], F32, kind="ExternalInput")
    out_d = nc.dram_tensor("out", [1, BC], F32, kind="ExternalOutput")

    NW = min(512, BC)
    n_nt = BC // NW

    with tile.TileContext(nc) as tc:
        with (
            tc.tile_pool(name="const", bufs=1) as cpool,
            tc.tile_pool(name="ly", bufs=1) as lypool,
            tc.tile_pool(name="gat", bufs=4) as gpool,
            tc.tile_pool(name="gram", bufs=2) as grpool,
            tc.tile_pool(name="tch", bufs=1) as tpool,
            tc.tile_pool(name="act", bufs=2) as apool,
            tc.tile_pool(name="outp", bufs=1) as opool,
            tc.tile_pool(name="pps", bufs=2, space="PSUM") as ppsum,
            tc.tile_pool(name="grps", bufs=4, space="PSUM") as grpsum,
            tc.tile_pool(name="mps", bufs=2, space="PSUM") as mpsum,
        ):
            offs = cpool.tile([128, n_tiles * NT * P], I32, tag="offs")
            nc.sync.dma_start(offs[:], offs_d[:])
            pmat = cpool.tile([128, P, 128], BF16, tag="pmat")
            nc.sync.dma_start(pmat[:], pmat_d[:])
            xT = cpool.tile([13, BC], BF16, tag="xT")
            nc.sync.dma_start(xT[:], xT_d[:])
            wb0 = cpool.tile([13, 512], BF16, tag="wb0")
            nc.sync.dma_start(wb0[:], wb0_d[:])
            wb1 = cpool.tile([128, 4, 256], BF16, tag="wb1")
            nc.sync.dma_start(wb1[:], wb1_d[:])
            wb2 = cpool.tile([128, 2, 128], BF16, tag="wb2")
            nc.sync.dma_start(wb2[:], wb2_d[:])
            tw0 = cpool.tile([128, 9, 1024], BF16, tag="tw0")
            nc.sync.dma_start(tw0[:], tw0_d[:])
            tw1 = cpool.tile([128, 8, 1024], BF16, tag="tw1")
            nc.sync.dma_start(tw1[:], tw1_d[:])
            tw2 = cpool.tile([128, 8, 512], BF16, tag="tw2")
            nc.sync.dma_start(tw2[:], tw2_d[:])
            tw3 = cpool.tile([128, 4, 256], BF16, tag="tw3")
            nc.sync.dma_start(tw3[:], tw3_d[:])
            tw4 = cpool.tile([128, 2, 1], BF16, tag="tw4")
            nc.sync.dma_start(tw4[:], tw4_d[:])
            bias = cpool.tile([128, 30], F32, tag="bias")
            nc.sync.dma_start(bias[:], bias_d[:])

            def bias_ap(col):
                return bias[:, col:col + 1]

            for _round in range(rounds):
                # LY: pooled features [128 D, tile, feat, 128 bags] bf16
                LY = lypool.tile([128, n_tiles, NF, 128], BF16, tag="LY")
                T = tpool.tile([128, NCHUNK, BC], BF16, tag="T")
                nc.vector.memset(T[:], 0.0)

                # ---- bottom MLP
                hb0 = apool.tile([128, 4, BC], BF16, tag="hact")
                for mt in range(4):
                    for ntl in range(n_nt):
                        ps = mpsum.tile([128, NW], F32, tag="mlp")
                        nc.tensor.matmul(ps[:], wb0[:, mt * 128:(mt + 1) * 128],
                                         xT[:, ntl * NW:(ntl + 1) * NW],
                                         start=True, stop=True)
                        nc.scalar.activation(hb0[:, mt, ntl * NW:(ntl + 1) * NW],
                                             ps[:], AF.Relu, bias=bias_ap(0 + mt))
                hb1 = apool.tile([128, 2, BC], BF16, tag="hact")
                for mt in range(2):
                    for ntl in range(n_nt):
                        ps = mpsum.tile([128, NW], F32, tag="mlp")
                        for kc in range(4):
                            nc.tensor.matmul(ps[:], wb1[:, kc, mt * 128:(mt + 1) * 128],
                                             hb0[:, kc, ntl * NW:(ntl + 1) * NW],
                                             start=(kc == 0), stop=(kc == 3))
                        nc.scalar.activation(hb1[:, mt, ntl * NW:(ntl + 1) * NW],
                                             ps[:], AF.Relu, bias=bias_ap(4 + mt))
                for ntl in range(n_nt):
                    ps = mpsum.tile([128, NW], F32, tag="mlp")
                    for kc in range(2):
                        nc.tensor.matmul(ps[:], wb2[:, kc, :],
                                         hb1[:, kc, ntl * NW:(ntl + 1) * NW],
                                         start=(kc == 0), stop=(kc == 1))
                    tpt = NW // 128
                    nc.scalar.activation(LY[:, ntl * tpt:(ntl + 1) * tpt, 0, :],
                                         ps[:], AF.Relu, bias=bias_ap(6))

                # ---- gather + pool + gram + extract, per 128-bag tile
                for tl in range(n_tiles):
                    for tgrp in range(0, NT, 4):
                        gsz = min(4, NT - tgrp)
                        pps = ppsum.tile([128, 512], F32, tag="pool")
                        for tt in range(tgrp, tgrp + gsz):
                            tloc = tt - tgrp
                            s = tl * NT + tt
                            G = gpool.tile([128, P, D], BF16, tag="G")
                            # one row per partition per instruction (the only
                            # indirect-gather shape real HW honors)
                            for k in range(P):
                                nc.gpsimd.indirect_dma_start(
                                    out=G[:, k, :], out_offset=None,
                                    in_=emb[:],
                                    in_offset=bass.IndirectOffsetOnAxis(
                                        ap=offs[:, s * P + k:s * P + k + 1], axis=0),
                                )
                            # slot 0 writes the full 128-col range (pmat is
                            # zero outside its bags) -> clears + initializes
                            nc.tensor.matmul(pps[:, tloc * 128:(tloc + 1) * 128],
                                             G[:, 0, :], pmat[:, 0, :],
                                             start=True, stop=False,
                                             skip_group_check=True)
                            for k in range(1, P):
                                lo = (k * 128) // P
                                hi = (k * 128 + 127) // P
                                nc.tensor.matmul(
                                    pps[:, tloc * 128 + lo: tloc * 128 + hi + 1],
                                    G[:, k, :], pmat[:, k, lo:hi + 1],
                                    start=False, stop=(k == P - 1),
                                    skip_group_check=True)
                        nc.scalar.activation(LY[:, tl, 1 + tgrp: 1 + tgrp + gsz, :],
                                             pps[:, :gsz * 128], AF.Copy)

                    GR = grpool.tile([27, 128 * NF], BF16, tag="GR")
                    for bb in range(8):
                        gps = grpsum.tile([27, 16 * NF], F32, tag="grps")
                        for j in range(16):
                            b = bb * 16 + j
                            fb = LY[:, tl, :, b]
                            nc.tensor.matmul(gps[:, j * NF:(j + 1) * NF], fb, fb,
                                             start=True, stop=True)
                        nc.scalar.activation(GR[:, bb * 16 * NF:(bb + 1) * 16 * NF],
                                             gps[:], AF.Copy)
                    grv = GR[:].rearrange("p (b m) -> p b m", m=NF)
                    for g in range(NCHUNK):
                        for mi in range(4):
                            m = 4 * g + mi
                            if m >= NF:
                                continue
                            nc.vector.tensor_copy(
                                T[32 * mi:32 * mi + 27, g, tl * 128:(tl + 1) * 128],
                                grv[0:27, :, m])

                # ---- top MLP
                h1 = apool.tile([128, 8, BC], BF16, tag="hact")
                for mt in range(8):
                    for ntl in range(n_nt):
                        ps = mpsum.tile([128, NW], F32, tag="mlp")
                        tpt = NW // 128
                        nc.tensor.matmul(ps[:], tw0[:, 0, mt * 128:(mt + 1) * 128],
                                         LY[:, ntl * tpt:(ntl + 1) * tpt, 0, :],
                                         start=True, stop=False)
                        for kc in range(1, 8):
                            nc.tensor.matmul(ps[:], tw0[:, kc, mt * 128:(mt + 1) * 128],
                                             T[:, kc - 1, ntl * NW:(ntl + 1) * NW],
                                             start=False, stop=(kc == 7))
                        nc.scalar.activation(h1[:, mt, ntl * NW:(ntl + 1) * NW],
                                             ps[:], AF.Relu, bias=bias_ap(7 + mt))
                h2 = apool.tile([128, 8, BC], BF16, tag="hact")
                for mt in range(8):
                    for ntl in range(n_nt):
                        ps = mpsum.tile([128, NW], F32, tag="mlp")
                        for kc in range(8):
                            nc.tensor.matmul(ps[:], tw1[:, kc, mt * 128:(mt + 1) * 128],
                                             h1[:, kc, ntl * NW:(ntl + 1) * NW],
                                             start=(kc == 0), stop=(kc == 7))
                        nc.scalar.activation(h2[:, mt, ntl * NW:(ntl + 1) * NW],
                                             ps[:], AF.Relu, bias=bias_ap(15 + mt))
                h3 = apool.tile([128, 4, BC], BF16, tag="hact")
                for mt in range(4):
                    for ntl in range(n_nt):
                        ps = mpsum.tile([128, NW], F32, tag="mlp")
                        for kc in range(8):
                            nc.tensor.matmul(ps[:], tw2[:, kc, mt * 128:(mt + 1) * 128],
                                             h2[:, kc, ntl * NW:(ntl + 1) * NW],
                                             start=(kc == 0), stop=(kc == 7))
                        nc.scalar.activation(h3[:, mt, ntl * NW:(ntl + 1) * NW],
                                             ps[:], AF.Relu, bias=bias_ap(23 + mt))
                h4 = apool.tile([128, 2, BC], BF16, tag="hact")
                for mt in range(2):
                    for ntl in range(n_nt):
                        ps = mpsum.tile([128, NW], F32, tag="mlp")
                        for kc in range(4):
                            nc.tensor.matmul(ps[:], tw3[:, kc, mt * 128:(mt + 1) * 128],
                                             h3[:, kc, ntl * NW:(ntl + 1) * NW],
                                             start=(kc == 0), stop=(kc == 3))
                        nc.scalar.activation(h4[:, mt, ntl * NW:(ntl + 1) * NW],
                                             ps[:], AF.Relu, bias=bias_ap(27 + mt))
                osig = opool.tile([1, BC], F32, tag="osig")
                for ntl in range(n_nt):
                    ps = mpsum.tile([1, NW], F32, tag="mlp")
                    for kc in range(2):
                        nc.tensor.matmul(ps[:], tw4[:, kc, :],
                                         h4[:, kc, ntl * NW:(ntl + 1) * NW],
                                         start=(kc == 0), stop=(kc == 1))
                    nc.scalar.activation(osig[:, ntl * NW:(ntl + 1) * NW], ps[:],
                                         AF.Sigmoid, bias=bias[0:1, 29:30])
                nc.sync.dma_start(out_d[:], osig[:])

    nc.compile()
    return nc


# =============================== host prep ===============================

def _bf(a):
    return np.ascontiguousarray(a.astype(ml_dtypes.bfloat16))


def _chunk_lhsT(w, n_kc):
    K, M = w.shape
    assert K == n_kc * 128
    return np.ascontiguousarray(w.reshape(n_kc, 128, M).transpose(1, 0, 2))


def prep_shared(inputs, vocab=VOCAB):
    out = {}
    out["emb"] = np.ascontiguousarray(
        np.asarray(inputs["emb_tables"], np.float32).reshape(NT * vocab, D))

    pmat = np.zeros((128, P, 128), np.float32)
    for k in range(P):
        for p in range(128):
            pmat[p, k, (k * 128 + p) // P] = 1.0
    out["pmat"] = _bf(pmat)

    out["wb0"] = _bf(np.asarray(inputs["bot_w0"], np.float32).T)
    out["wb1"] = _bf(_chunk_lhsT(np.asarray(inputs["bot_w1"], np.float32).T, 4))
    out["wb2"] = _bf(_chunk_lhsT(np.asarray(inputs["bot_w2"], np.float32).T, 2))

    W0 = np.asarray(inputs["top_w0"], np.float32)
    li, lj = np.triu_indices(NF, k=1)
    paircol = np.zeros((NF, NF), np.int64)
    for i, (a, b) in enumerate(zip(li, lj)):
        paircol[a, b] = paircol[b, a] = 128 + i
    W0p = np.zeros((TOPK0, 1024), np.float32)
    W0p[0:128] = W0[:, 0:128].T
    for g in range(NCHUNK):
        for mi in range(4):
            m = 4 * g + mi
            if m >= NF:
                continue
            for n in range(NF):
                if n == m:
                    continue
                W0p[128 + 128 * g + 32 * mi + n] = 0.5 * W0[:, paircol[n, m]]
    out["tw0"] = _bf(_chunk_lhsT(W0p, 9))
    out["tw1"] = _bf(_chunk_lhsT(np.asarray(inputs["top_w1"], np.float32).T, 8))
    out["tw2"] = _bf(_chunk_lhsT(np.asarray(inputs["top_w2"], np.float32).T, 8))
    out["tw3"] = _bf(_chunk_lhsT(np.asarray(inputs["top_w3"], np.float32).T, 4))
    out["tw4"] = _bf(_chunk_lhsT(np.asarray(inputs["top_w4"], np.float32).T, 2))

    bias = np.zeros((128, 30), np.float32)
    specs = [("bot_b0", 0, 4), ("bot_b1", 4, 2), ("bot_b2", 6, 1),
             ("top_b0", 7, 8), ("top_b1", 15, 8), ("top_b2", 23, 4),
             ("top_b3", 27, 2), ("top_b4", 29, 1)]
    for name, c0, nmt in specs:
        b = np.asarray(inputs[name], np.float32)
        bp = np.zeros(nmt * 128, np.float32)
        bp[:b.shape[0]] = b
        bias[:, c0:c0 + nmt] = bp.reshape(nmt, 128).T
    out["bias"] = bias
    return out


def prep_core(inputs, core, vocab=VOCAB, n_tiles=N_TILES):
    BC = n_tiles * 128
    out = {}
    dx = np.asarray(inputs["dense_x"], np.float32)[core * BC:(core + 1) * BC]
    out["xT"] = _bf(dx.T)

    B_total = np.asarray(inputs["sparse_indices"]).shape[1] // P
    idx = np.asarray(inputs["sparse_indices"], np.int64).reshape(NT, B_total, P)
    idx = idx[:, core * BC:(core + 1) * BC, :]
    lin = idx.reshape(NT, n_tiles, 128 * P)          # l = bag_in_tile*P + j
    arr = lin.reshape(NT, n_tiles, P, 128)           # [t, tile, k, p]; l = k*128+p
    arr = arr + (np.arange(NT) * vocab)[:, None, None, None]
    off = arr.transpose(3, 1, 0, 2).reshape(128, n_tiles * NT * P)
    out["offs"] = np.ascontiguousarray(off.astype(np.int32))
    return out


def make_in_maps(inputs, vocab=VOCAB, n_tiles=N_TILES, n_cores=N_CORES):
    shared = prep_shared(inputs, vocab)
    maps = []
    for c in range(n_cores):
        m = dict(shared)
        m.update(prep_core(inputs, c, vocab, n_tiles))
        maps.append(m)
    return maps


_NC_CACHE = {}


def _get_nc(rounds=1):
    key = rounds
    if key not in _NC_CACHE:
        _NC_CACHE[key] = build_nc(VOCAB, N_TILES, rounds)
    return _NC_CACHE[key]


def kernel(**inputs) -> np.ndarray:
    from concourse.bass_utils import run_bass_kernel_spmd
    nc = _get_nc()
    maps = make_in_maps(inputs)
    res = run_bass_kernel_spmd(nc, maps, core_ids=list(range(N_CORES)))
    out = np.concatenate([np.asarray(res.results[c]["out"]).reshape(-1)
                          for c in range(N_CORES)])
    return np.ascontiguousarray(out.astype(np.float32))



# revision 4
# speedup vs baseline: 1.0685x; 1.0685x over previous
"""DLRM on 8 Trainium2 NeuronCores (Bass/Tile), batch-split data-parallel.

Strategy: each core handles 1024 of the 8192 samples end-to-end; the 26
embedding tables are replicated to every core's HBM, so no collectives are
needed.  Per core: bottom MLP (bf16 matmuls, f32 PSUM) -> indirect-DMA
gather of embedding rows with inline f32->bf16 cast -> sum-pooling as PE
matmuls against a 0/1 pooling matrix (gives feature-major pooled vectors
directly) -> per-sample 27x27 gram matmuls -> DVE extraction into
32-aligned pair-chunk tiles -> top MLP with a host-transformed first-layer
weight (symmetric-half trick replaces the upper-triangle gather) ->
sigmoid -> [1024] f32.
"""
import sys
import numpy as np

if '/opt/trn_rl_repo' not in sys.path:
    sys.path.insert(0, '/opt/trn_rl_repo')

import ml_dtypes
import concourse.bass as bass
import concourse.mybir as mybir
import concourse.tile as tile
from concourse import bacc

F32 = mybir.dt.float32
BF16 = mybir.dt.bfloat16
I32 = mybir.dt.int32
AF = mybir.ActivationFunctionType

NT = 26          # tables
NF = 27          # features entering interaction
D = 128
P = 10           # pooling factor
VOCAB = 100000
N_TILES = 8      # 128-bag tiles per core
N_CORES = 8
TOPK0 = 1152     # padded z' rows (128 x + 7*128 pair chunks)
NCHUNK = 7


def build_nc(vocab: int = VOCAB, n_tiles: int = N_TILES, rounds: int = 1):
    BC = n_tiles * 128
    nc = bacc.Bacc(None, target_bir_lowering=False)

    emb = nc.dram_tensor("emb", [NT * vocab, D], F32, kind="ExternalInput")
    offs_d = nc.dram_tensor("offs", [128, n_tiles * NT * P], I32, kind="ExternalInput")
    xT_d = nc.dram_tensor("xT", [13, BC], BF16, kind="ExternalInput")
    pmat_d = nc.dram_tensor("pmat", [128, P, 128], BF16, kind="ExternalInput")
    wb0_d = nc.dram_tensor("wb0", [13, 512], BF16, kind="ExternalInput")
    wb1_d = nc.dram_tensor("wb1", [128, 4, 256], BF16, kind="ExternalInput")
    wb2_d = nc.dram_tensor("wb2", [128, 2, 128], BF16, kind="ExternalInput")
    tw0_d = nc.dram_tensor("tw0", [128, 9, 1024], BF16, kind="ExternalInput")
    tw1_d = nc.dram_tensor("tw1", [128, 8, 1024], BF16, kind="ExternalInput")
    tw2_d = nc.dram_tensor("tw2", [128, 8, 512], BF16, kind="ExternalInput")
    tw3_d = nc.dram_tensor("tw3", [128, 4, 256], BF16, kind="ExternalInput")
    tw4_d = nc.dram_tensor("tw4", [128, 2, 1], BF16, kind="ExternalInput")
    bias_d = nc.dram_tensor("bias", [128, 30], F32, kind="ExternalInput")
    out_d = nc.dram_tensor("out", [1, BC], F32, kind="ExternalOutput")

    NW = min(512, BC)
    n_nt = BC // NW

    with tile.TileContext(nc) as tc:
        with (
            tc.tile_pool(name="const", bufs=1) as cpool,
            tc.tile_pool(name="ly", bufs=1) as lypool,
            tc.tile_pool(name="gat", bufs=8) as gpool,
            tc.tile_pool(name="gram", bufs=2) as grpool,
            tc.tile_pool(name="tch", bufs=1) as tpool,
            tc.tile_pool(name="act", bufs=2) as apool,
            tc.tile_pool(name="outp", bufs=1) as opool,
            tc.tile_pool(name="pps", bufs=2, space="PSUM") as ppsum,
            tc.tile_pool(name="grps", bufs=4, space="PSUM") as grpsum,
            tc.tile_pool(name="mps", bufs=2, space="PSUM") as mpsum,
        ):
            offs = cpool.tile([128, n_tiles * NT * P], I32, tag="offs")
            nc.sync.dma_start(offs[:], offs_d[:])
            pmat = cpool.tile([128, P, 128], BF16, tag="pmat")
            nc.sync.dma_start(pmat[:], pmat_d[:])
            xT = cpool.tile([13, BC], BF16, tag="xT")
            nc.sync.dma_start(xT[:], xT_d[:])
            wb0 = cpool.tile([13, 512], BF16, tag="wb0")
            nc.sync.dma_start(wb0[:], wb0_d[:])
            wb1 = cpool.tile([128, 4, 256], BF16, tag="wb1")
            nc.sync.dma_start(wb1[:], wb1_d[:])
            wb2 = cpool.tile([128, 2, 128], BF16, tag="wb2")
            nc.sync.dma_start(wb2[:], wb2_d[:])
            tw0 = cpool.tile([128, 9, 1024], BF16, tag="tw0")
            nc.sync.dma_start(tw0[:], tw0_d[:])
            tw1 = cpool.tile([128, 8, 1024], BF16, tag="tw1")
            nc.sync.dma_start(tw1[:], tw1_d[:])
            tw2 = cpool.tile([128, 8, 512], BF16, tag="tw2")
            nc.sync.dma_start(tw2[:], tw2_d[:])
            tw3 = cpool.tile([128, 4, 256], BF16, tag="tw3")
            nc.sync.dma_start(tw3[:], tw3_d[:])
            tw4 = cpool.tile([128, 2, 1], BF16, tag="tw4")
            nc.sync.dma_start(tw4[:], tw4_d[:])
            bias = cpool.tile([128, 30], F32, tag="bias")
            nc.sync.dma_start(bias[:], bias_d[:])

            def bias_ap(col):
                return bias[:, col:col + 1]

            for _round in range(rounds):
                # LY: pooled features [128 D, tile, feat, 128 bags] bf16
                LY = lypool.tile([128, n_tiles, NF, 128], BF16, tag="LY")
                T = tpool.tile([128, NCHUNK, BC], BF16, tag="T")
                nc.vector.memset(T[:], 0.0)

                # ---- bottom MLP
                hb0 = apool.tile([128, 4, BC], BF16, tag="hact")
                for mt in range(4):
                    for ntl in range(n_nt):
                        ps = mpsum.tile([128, NW], F32, tag="mlp")
                        nc.tensor.matmul(ps[:], wb0[:, mt * 128:(mt + 1) * 128],
                                         xT[:, ntl * NW:(ntl + 1) * NW],
                                         start=True, stop=True)
                        nc.scalar.activation(hb0[:, mt, ntl * NW:(ntl + 1) * NW],
                                             ps[:], AF.Relu, bias=bias_ap(0 + mt))
                hb1 = apool.tile([128, 2, BC], BF16, tag="hact")
                for mt in range(2):
                    for ntl in range(n_nt):
                        ps = mpsum.tile([128, NW], F32, tag="mlp")
                        for kc in range(4):
                            nc.tensor.matmul(ps[:], wb1[:, kc, mt * 128:(mt + 1) * 128],
                                             hb0[:, kc, ntl * NW:(ntl + 1) * NW],
                                             start=(kc == 0), stop=(kc == 3))
                        nc.scalar.activation(hb1[:, mt, ntl * NW:(ntl + 1) * NW],
                                             ps[:], AF.Relu, bias=bias_ap(4 + mt))
                for ntl in range(n_nt):
                    ps = mpsum.tile([128, NW], F32, tag="mlp")
                    for kc in range(2):
                        nc.tensor.matmul(ps[:], wb2[:, kc, :],
                                         hb1[:, kc, ntl * NW:(ntl + 1) * NW],
                                         start=(kc == 0), stop=(kc == 1))
                    tpt = NW // 128
                    nc.scalar.activation(LY[:, ntl * tpt:(ntl + 1) * tpt, 0, :],
                                         ps[:], AF.Relu, bias=bias_ap(6))

                osig = opool.tile([1, BC], F32, tag="osig")

                def emit_top_block(ntl):
                    """Top MLP for column block ntl (tiles ntl*tpt..(ntl+1)*tpt-1).

                    Emitted as soon as those tiles' T chunks are written so the
                    top MLP overlaps the remaining tiles' gathers instead of
                    trailing after the last one."""
                    tpt = NW // 128
                    c0, c1 = ntl * NW, (ntl + 1) * NW
                    h1 = apool.tile([128, 8, NW], BF16, tag="hact")
                    for mt in range(8):
                        ps = mpsum.tile([128, NW], F32, tag="mlp")
                        nc.tensor.matmul(ps[:], tw0[:, 0, mt * 128:(mt + 1) * 128],
                                         LY[:, ntl * tpt:(ntl + 1) * tpt, 0, :],
                                         start=True, stop=False)
                        for kc in range(1, 8):
                            nc.tensor.matmul(ps[:], tw0[:, kc, mt * 128:(mt + 1) * 128],
                                             T[:, kc - 1, c0:c1],
                                             start=False, stop=(kc == 7))
                        nc.scalar.activation(h1[:, mt, :], ps[:], AF.Relu,
                                             bias=bias_ap(7 + mt))
                    h2 = apool.tile([128, 8, NW], BF16, tag="hact")
                    for mt in range(8):
                        ps = mpsum.tile([128, NW], F32, tag="mlp")
                        for kc in range(8):
                            nc.tensor.matmul(ps[:], tw1[:, kc, mt * 128:(mt + 1) * 128],
                                             h1[:, kc, :],
                                             start=(kc == 0), stop=(kc == 7))
                        nc.scalar.activation(h2[:, mt, :], ps[:], AF.Relu,
                                             bias=bias_ap(15 + mt))
                    h3 = apool.tile([128, 4, NW], BF16, tag="hact")
                    for mt in range(4):
                        ps = mpsum.tile([128, NW], F32, tag="mlp")
                        for kc in range(8):
                            nc.tensor.matmul(ps[:], tw2[:, kc, mt * 128:(mt + 1) * 128],
                                             h2[:, kc, :],
                                             start=(kc == 0), stop=(kc == 7))
                        nc.scalar.activation(h3[:, mt, :], ps[:], AF.Relu,
                                             bias=bias_ap(23 + mt))
                    h4 = apool.tile([128, 2, NW], BF16, tag="hact")
                    for mt in range(2):
                        ps = mpsum.tile([128, NW], F32, tag="mlp")
                        for kc in range(4):
                            nc.tensor.matmul(ps[:], tw3[:, kc, mt * 128:(mt + 1) * 128],
                                             h3[:, kc, :],
                                             start=(kc == 0), stop=(kc == 3))
                        nc.scalar.activation(h4[:, mt, :], ps[:], AF.Relu,
                                             bias=bias_ap(27 + mt))
                    ps = mpsum.tile([1, NW], F32, tag="mlp")
                    for kc in range(2):
                        nc.tensor.matmul(ps[:], tw4[:, kc, :], h4[:, kc, :],
                                         start=(kc == 0), stop=(kc == 1))
                    nc.scalar.activation(osig[:, c0:c1], ps[:],
                                         AF.Sigmoid, bias=bias[0:1, 29:30])

                # ---- gather + pool + gram + extract, per 128-bag tile
                for tl in range(n_tiles):
                    for tgrp in range(0, NT, 4):
                        gsz = min(4, NT - tgrp)
                        pps = ppsum.tile([128, 512], F32, tag="pool")
                        for tt in range(tgrp, tgrp + gsz):
                            tloc = tt - tgrp
                            s = tl * NT + tt
                            G = gpool.tile([128, P, D], BF16, tag="G")
                            # one row per partition per instruction (the only
                            # indirect-gather shape real HW honors)
                            for k in range(P):
                                nc.gpsimd.indirect_dma_start(
                                    out=G[:, k, :], out_offset=None,
                                    in_=emb[:],
                                    in_offset=bass.IndirectOffsetOnAxis(
                                        ap=offs[:, s * P + k:s * P + k + 1], axis=0),
                                )
                            # slot 0 writes the full 128-col range (pmat is
                            # zero outside its bags) -> clears + initializes
                            nc.tensor.matmul(pps[:, tloc * 128:(tloc + 1) * 128],
                                             G[:, 0, :], pmat[:, 0, :],
                                             start=True, stop=False,
                                             skip_group_check=True)
                            for k in range(1, P):
                                lo = (k * 128) // P
                                hi = (k * 128 + 127) // P
                                nc.tensor.matmul(
                                    pps[:, tloc * 128 + lo: tloc * 128 + hi + 1],
                                    G[:, k, :], pmat[:, k, lo:hi + 1],
                                    start=False, stop=(k == P - 1),
                                    skip_group_check=True)
                        nc.scalar.activation(LY[:, tl, 1 + tgrp: 1 + tgrp + gsz, :],
                                             pps[:, :gsz * 128], AF.Copy)

                    GR = grpool.tile([27, 128 * NF], BF16, tag="GR")
                    for bb in range(8):
                        gps = grpsum.tile([27, 16 * NF], F32, tag="grps")
                        for j in range(16):
                            b = bb * 16 + j
                            fb = LY[:, tl, :, b]
                            nc.tensor.matmul(gps[:, j * NF:(j + 1) * NF], fb, fb,
                                             start=True, stop=True)
                        nc.scalar.activation(GR[:, bb * 16 * NF:(bb + 1) * 16 * NF],
                                             gps[:], AF.Copy)
                    grv = GR[:].rearrange("p (b m) -> p b m", m=NF)
                    for g in range(NCHUNK):
                        for mi in range(4):
                            m = 4 * g + mi
                            if m >= NF:
                                continue
                            nc.vector.tensor_copy(
                                T[32 * mi:32 * mi + 27, g, tl * 128:(tl + 1) * 128],
                                grv[0:27, :, m])

                    # top MLP for a finished column block rides under the
                    # remaining tiles' gathers
                    tpt = NW // 128
                    if (tl + 1) % tpt == 0:
                        emit_top_block((tl + 1) // tpt - 1)

                nc.sync.dma_start(out_d[:], osig[:])

    nc.compile()
    return nc


# =============================== host prep ===============================

def _bf(a):
    return np.ascontiguousarray(a.astype(ml_dtypes.bfloat16))


def _chunk_lhsT(w, n_kc):
    K, M = w.shape
    assert K == n_kc * 128
    return np.ascontiguousarray(w.reshape(n_kc, 128, M).transpose(1, 0, 2))


def prep_shared(inputs, vocab=VOCAB):
    out = {}
    out["emb"] = np.ascontiguousarray(
        np.asarray(inputs["emb_tables"], np.float32).reshape(NT * vocab, D))

    pmat = np.zeros((128, P, 128), np.float32)
    for k in range(P):
        for p in range(128):
            pmat[p, k, (k * 128 + p) // P] = 1.0
    out["pmat"] = _bf(pmat)

    out["wb0"] = _bf(np.asarray(inputs["bot_w0"], np.float32).T)
    out["wb1"] = _bf(_chunk_lhsT(np.asarray(inputs["bot_w1"], np.float32).T, 4))
    out["wb2"] = _bf(_chunk_lhsT(np.asarray(inputs["bot_w2"], np.float32).T, 2))

    W0 = np.asarray(inputs["top_w0"], np.float32)
    li, lj = np.triu_indices(NF, k=1)
    paircol = np.zeros((NF, NF), np.int64)
    for i, (a, b) in enumerate(zip(li, lj)):
        paircol[a, b] = paircol[b, a] = 128 + i
    W0p = np.zeros((TOPK0, 1024), np.float32)
    W0p[0:128] = W0[:, 0:128].T
    for g in range(NCHUNK):
        for mi in range(4):
            m = 4 * g + mi
            if m >= NF:
                continue
            for n in range(NF):
                if n == m:
                    continue
                W0p[128 + 128 * g + 32 * mi + n] = 0.5 * W0[:, paircol[n, m]]
    out["tw0"] = _bf(_chunk_lhsT(W0p, 9))
    out["tw1"] = _bf(_chunk_lhsT(np.asarray(inputs["top_w1"], np.float32).T, 8))
    out["tw2"] = _bf(_chunk_lhsT(np.asarray(inputs["top_w2"], np.float32).T, 8))
    out["tw3"] = _bf(_chunk_lhsT(np.asarray(inputs["top_w3"], np.float32).T, 4))
    out["tw4"] = _bf(_chunk_lhsT(np.asarray(inputs["top_w4"], np.float32).T, 2))

    bias = np.zeros((128, 30), np.float32)
    specs = [("bot_b0", 0, 4), ("bot_b1", 4, 2), ("bot_b2", 6, 1),
             ("top_b0", 7, 8), ("top_b1", 15, 8), ("top_b2", 23, 4),
             ("top_b3", 27, 2), ("top_b4", 29, 1)]
    for name, c0, nmt in specs:
        b = np.asarray(inputs[name], np.float32)
        bp = np.zeros(nmt * 128, np.float32)
        bp[:b.shape[0]] = b
        bias[:, c0:c0 + nmt] = bp.reshape(nmt, 128).T
    out["bias"] = bias
    return out


def prep_core(inputs, core, vocab=VOCAB, n_tiles=N_TILES):
    BC = n_tiles * 128
    out = {}
    dx = np.asarray(inputs["dense_x"], np.float32)[core * BC:(core + 1) * BC]
    out["xT"] = _bf(dx.T)

    B_total = np.asarray(inputs["sparse_indices"]).shape[1] // P
    idx = np.asarray(inputs["sparse_indices"], np.int64).reshape(NT, B_total, P)
    idx = idx[:, core * BC:(core + 1) * BC, :]
    lin = idx.reshape(NT, n_tiles, 128 * P)          # l = bag_in_tile*P + j
    arr = lin.reshape(NT, n_tiles, P, 128)           # [t, tile, k, p]; l = k*128+p
    arr = arr + (np.arange(NT) * vocab)[:, None, None, None]
    off = arr.transpose(3, 1, 0, 2).reshape(128, n_tiles * NT * P)
    out["offs"] = np.ascontiguousarray(off.astype(np.int32))
    return out


def make_in_maps(inputs, vocab=VOCAB, n_tiles=N_TILES, n_cores=N_CORES):
    shared = prep_shared(inputs, vocab)
    maps = []
    for c in range(n_cores):
        m = dict(shared)
        m.update(prep_core(inputs, c, vocab, n_tiles))
        maps.append(m)
    return maps


_NC_CACHE = {}


def _get_nc(rounds=1):
    key = rounds
    if key not in _NC_CACHE:
        _NC_CACHE[key] = build_nc(VOCAB, N_TILES, rounds)
    return _NC_CACHE[key]


def kernel(**inputs) -> np.ndarray:
    from concourse.bass_utils import run_bass_kernel_spmd
    nc = _get_nc()
    maps = make_in_maps(inputs)
    res = run_bass_kernel_spmd(nc, maps, core_ids=list(range(N_CORES)))
    out = np.concatenate([np.asarray(res.results[c]["out"]).reshape(-1)
                          for c in range(N_CORES)])
    return np.ascontiguousarray(out.astype(np.float32))

